# revision 10
# baseline (speedup 1.0000x reference)
"""Trainium2 Bass kernel for a top-2 ternary-weight MoE FFN — v3.

Sharding: tensor-parallel over the expert hidden dim H instead of
expert-parallel. Every core holds a 256-row H-slice of ALL 8 experts'
ternary weights and processes ALL ~16.4k token-expert pairs (grouped by
expert, each segment padded to a uniform per-expert tile size). This is
perfectly load-balanced for any routing skew: per-core PE work is
pairs x 48 cycles ~= 328 us, vs padding every core to the hottest
expert (2176 x 384 cycles = 348 us) under expert-parallel.

Host: fp64 routing (decision-exact vs the fp32 reference ordering),
ternarization (exact median threshold, {-1,0,+1} exact in bf16), pair
gather, DMA-friendly layouts. Unshard: sum the 8 partial y (each core's
down matmul contracts only its 256 H rows), then combine-weighted
scatter-add per token.
"""

import os

import numpy as np
import ml_dtypes

import concourse.bacc as bacc
import concourse.mybir as mybir
from concourse.tile import TileContext
from concourse.bass_utils import run_bass_kernel_spmd

FP32 = mybir.dt.float32
BF16 = mybir.dt.bfloat16
NP_BF16 = ml_dtypes.bfloat16

NCORES = 8
B, T, D, H, E = 4, 2048, 1024, 2048, 8
N = B * T                    # 8192 tokens
KO_D = 8                     # contraction chunks over D
HSH = H // NCORES            # 256 H rows per core per expert
KL = HSH // 128              # 2 local h chunks per expert
WSLAB = 256
DB = D // WSLAB              # 4 down-weight slabs

LAST_HW_NS = None
LAST_PHASE_NS = None

_program_cache = {}


def _ensure_ntff_hook():
    """Profiling-only: register the axon NTFF hook that the trimmed antenv
    package lacks, and stub out artifact upload (no bucket creds here)."""
    import sys
    import types

    import concourse.bass_utils as bu
    bu.upload_artifacts = lambda d: str(d)
    try:
        from antenv.axon_hooks import get_axon_ntff_profile_hook
        if get_axon_ntff_profile_hook() is not None:
            return
    except ImportError:
        mod = types.ModuleType("antenv.axon_hooks")
        box = {}
        mod.set_axon_ntff_profile_hook = lambda h: box.__setitem__("h", h)
        mod.get_axon_ntff_profile_hook = lambda: box.get("h")
        sys.modules["antenv.axon_hooks"] = mod
        import antenv
        antenv.axon_hooks = mod
    from antenv.axon_hooks import set_axon_ntff_profile_hook
    from trn_agent_boot.trn_boot import _ntff_profile_via_ctypes
    set_axon_ntff_profile_hook(
        _ntff_profile_via_ctypes("/opt/axon/libaxon_pjrt.so"))


def _run(nc, in_maps, label):
    trace = bool(int(os.environ.get("MOE_TRACE", "0")))
    kw = {}
    if trace:
        _ensure_ntff_hook()
        kw = dict(trace=True, trace_cores=list(range(NCORES)),
                  trace_kwargs={"title": label})
    res = run_bass_kernel_spmd(nc, in_maps, core_ids=list(range(NCORES)), **kw)
    if trace:
        global LAST_PHASE_NS
        print(f"[{label}] exec_time_ns={res.exec_time_ns} "
              f"mean={res.mean_exec_time_ns} "
              f"slowest_core={res.max_exec_time_core_id} "
              f"trace={res.instructions_and_trace[1] if res.instructions_and_trace else None}")
        if res.exec_time_ns:
            LAST_PHASE_NS[label] = res.exec_time_ns
    return res


def _tile_geom(mc):
    """Uniform tiles for one expert segment: nt tiles of tsz (<=512, x4)."""
    nt = max(1, -(-mc // 512))
    tsz = -(-mc // (nt * 4)) * 4
    return nt, tsz


def _build_ffn(geoms):
    """All-expert H-slice FFN. geoms[e] = (nt_e, tsz_e).

    DRAM layouts (per core, all bf16):
      wg/wu [E, 128, KO_D, 256]: [e, p, k, c] = tern(w_e).T[k*128+p, hsl+c]
        (hsl = this core's 256-column H-slice of expert e)
      wd    [DB, 128, E*KL, 256]: [j, p, e*2+l, c]
              = tern(wd_e).T[hsl + l*128 + p, j*256+c]
      xg_e  [nt_e, 128, KO_D, tsz_e]: expert-e pair tokens, tiled
      yt_e  [nt_e, 128, KO_D, tsz_e]: partial outputs (sum over cores)
    """
    nc = bacc.Bacc("TRN2", target_bir_lowering=False, debug=False,
                   num_devices=NCORES)
    wg = nc.dram_tensor("wg", [E, 128, KO_D, WSLAB], BF16,
                        kind="ExternalInput")
    wu = nc.dram_tensor("wu", [E, 128, KO_D, WSLAB], BF16,
                        kind="ExternalInput")
    wd = nc.dram_tensor("wd", [DB, 128, E * KL, WSLAB], BF16,
                        kind="ExternalInput")
    xg, yt = [], []
    for e, (nt, tsz) in enumerate(geoms):
        xg.append(nc.dram_tensor(f"xg{e}", [nt, 128, KO_D, tsz], BF16,
                                 kind="ExternalInput"))
        yt.append(nc.dram_tensor(f"yt{e}", [nt, 128, KO_D, tsz], BF16,
                                 kind="ExternalOutput"))

    with TileContext(nc) as tc:
        with (
            tc.tile_pool(name="wpool", bufs=1) as wpool,
            tc.tile_pool(name="xpool", bufs=2) as xpool,
            tc.tile_pool(name="mpool", bufs=2) as mpool,
            tc.tile_pool(name="spool", bufs=3) as spool,
            tc.tile_pool(name="ypool", bufs=4) as ypool,
            tc.tile_pool(name="ps_g", bufs=2, space="PSUM") as ps_g,
            tc.tile_pool(name="ps_u", bufs=2, space="PSUM") as ps_u,
            tc.tile_pool(name="ps_o", bufs=4, space="PSUM") as ps_o,
        ):
            # SBUF-resident H-slice weights: 24 KB/partition total.
            wg_sb = wpool.tile([128, E, KO_D, WSLAB], BF16)
            wu_sb = wpool.tile([128, E, KO_D, WSLAB], BF16)
            wd_sb = wpool.tile([128, DB, E * KL, WSLAB], BF16)

            def load_xt(e, ti, tsz, chunked):
                xt_sb = xpool.tile([128, KO_D, 512], BF16, tag="xt")
                if chunked:  # very first tile: per-k DMAs to start sooner
                    for k in range(KO_D):
                        nc.sync.dma_start(xt_sb[:, k, :tsz],
                                          xg[e].ap()[ti, :, k, :])
                else:
                    nc.sync.dma_start(xt_sb[:, :, :tsz], xg[e].ap()[ti])
                return xt_sb

            # tokens on the sync queue; ALL weights on the SWDGE queue in
            # exact consumption order (expert-interleaved gate/up, then
            # down). Expert e's wg+wu slabs (1 MiB) feed a whole ~35us
            # segment, so delivery stays far ahead of the PE. First slabs
            # halved so the first matmul chain starts sooner.
            xt_cur = load_xt(0, 0, geoms[0][1], True)
            nc.gpsimd.dma_start(wg_sb[:, 0, :, :128], wg.ap()[0, :, :, :128])
            nc.gpsimd.dma_start(wu_sb[:, 0, :, :128], wu.ap()[0, :, :, :128])
            nc.gpsimd.dma_start(wg_sb[:, 0, :, 128:], wg.ap()[0, :, :, 128:])
            nc.gpsimd.dma_start(wu_sb[:, 0, :, 128:], wu.ap()[0, :, :, 128:])
            for e in range(1, E):
                nc.gpsimd.dma_start(wg_sb[:, e], wg.ap()[e])
                nc.gpsimd.dma_start(wu_sb[:, e], wu.ap()[e])
            for j in range(DB):
                nc.gpsimd.dma_start(wd_sb[:, j], wd.ap()[j])
            drain_alt = 0
            for e in range(E):
                nt, tsz = geoms[e]
                for ti in range(nt):
                    m_sb = mpool.tile([128, KL, 512], BF16, tag="m")
                    for hl in range(KL):
                        pg = ps_g.tile([128, tsz], FP32, tag="pg")
                        pu = ps_u.tile([128, tsz], FP32, tag="pu")
                        csl = slice(hl * 128, (hl + 1) * 128)
                        for k in range(KO_D):
                            nc.tensor.matmul(pg[:], lhsT=wg_sb[:, e, k, csl],
                                             rhs=xt_cur[:, k, :tsz],
                                             start=(k == 0),
                                             stop=(k == KO_D - 1))
                        for k in range(KO_D):
                            nc.tensor.matmul(pu[:], lhsT=wu_sb[:, e, k, csl],
                                             rhs=xt_cur[:, k, :tsz],
                                             start=(k == 0),
                                             stop=(k == KO_D - 1))
                        sg = spool.tile([128, 512], BF16, tag="sg")
                        nc.scalar.activation(sg[:, :tsz], pg[:],
                                             mybir.ActivationFunctionType.Silu)
                        nc.vector.tensor_tensor(out=m_sb[:, hl, :tsz],
                                                in0=sg[:, :tsz], in1=pu[:],
                                                op=mybir.AluOpType.mult)
                    # prefetch next tile's tokens during the down matmuls
                    nxt = None
                    if ti + 1 < nt:
                        nxt = (e, ti + 1, tsz)
                    elif e + 1 < E:
                        nxt = (e + 1, 0, geoms[e + 1][1])
                    if nxt is not None:
                        xt_next = load_xt(nxt[0], nxt[1], nxt[2], False)
                    # whole-tile output staging: one store descriptor per
                    # tile keeps the SWDGE issue rate low (32 MiB total)
                    ysb = ypool.tile([128, KO_D, 512], BF16, tag="ysb")
                    for d in range(KO_D):
                        j, r = divmod(d, 2)
                        dsl = slice(r * 128, (r + 1) * 128)
                        po = ps_o.tile([128, tsz], FP32, tag="po")
                        for hl in range(KL):
                            nc.tensor.matmul(
                                po[:], lhsT=wd_sb[:, j, e * KL + hl, dsl],
                                rhs=m_sb[:, hl, :tsz],
                                start=(hl == 0), stop=(hl == KL - 1))
                        # short accumulation chains drain faster than one
                        # engine can copy: alternate ACT / DVE
                        if drain_alt % 2 == 0:
                            nc.scalar.copy(ysb[:, d, :tsz], po[:])
                        else:
                            nc.vector.tensor_copy(ysb[:, d, :tsz], po[:])
                        drain_alt += 1
                    nc.gpsimd.dma_start(yt[e].ap()[ti], ysb[:, :, :tsz])
                    if nxt is not None:
                        xt_cur = xt_next
    nc.compile()
    return nc


def _get_program(key):
    if key not in _program_cache:
        _program_cache[key] = _build_ffn(key)
    return _program_cache[key]


def _ternary_t(w):
    """tern(w).T int8; exact median-of-|w| threshold."""
    w = np.ascontiguousarray(w, dtype=np.float32)
    med = np.median(np.abs(w))
    q = (w > med).astype(np.int8) - (w < -med).astype(np.int8)
    return np.ascontiguousarray(q.T)


def kernel(x, router_w, w_gate, w_up, w_down, top_k):
    assert int(top_k) == 2
    global LAST_HW_NS, LAST_PHASE_NS
    LAST_PHASE_NS = {}
    xf = np.ascontiguousarray(x.reshape(N, D).astype(np.float32))

    # ---- host routing (fp64 logits; top-2 ordering matches the fp32
    # reference, gaps are far above fp32 rounding noise) ----
    logits = xf.astype(np.float64) @ router_w.T.astype(np.float64)
    order = np.argsort(-logits, axis=1, kind="stable")
    e1 = order[:, 0]
    e2 = order[:, 1]
    ar = np.arange(N)
    w1 = 1.0 / (1.0 + np.exp(-(logits[ar, e1] - logits[ar, e2])))
    w2 = 1.0 - w1

    # ---- pair lists per expert ----
    toks, wts = [], []
    for e in range(E):
        sel = np.nonzero((e1 == e) | (e2 == e))[0]
        toks.append(sel)
        wts.append(np.where(e1[sel] == e, w1[sel], w2[sel]).astype(np.float32))
    counts = [len(s) for s in toks]
    geoms = tuple(_tile_geom(c) for c in counts)

    fnc = _get_program(geoms)
    xf_bf = xf.astype(NP_BF16)

    # per-expert token tiles (identical on every core)
    xg_arrs = {}
    for e in range(E):
        nt, tsz = geoms[e]
        cap = nt * tsz
        xgp = np.zeros((cap, D), dtype=NP_BF16)
        xgp[:counts[e]] = xf_bf[toks[e]]
        xg_arrs[f"xg{e}"] = np.ascontiguousarray(
            xgp.reshape(nt, tsz, KO_D, 128).transpose(0, 3, 2, 1))

    # per-core H-slice weight slabs
    tg = [_ternary_t(w_gate[e]) for e in range(E)]   # [D, H] int8
    tu = [_ternary_t(w_up[e]) for e in range(E)]
    td = [_ternary_t(w_down[e]) for e in range(E)]   # [H, D] int8
    in_maps = []
    for c in range(NCORES):
        hsl = slice(c * HSH, (c + 1) * HSH)
        wg_c = np.stack([tg[e][:, hsl].reshape(KO_D, 128, WSLAB)
                         for e in range(E)])          # [E, k, p, c]
        wu_c = np.stack([tu[e][:, hsl].reshape(KO_D, 128, WSLAB)
                         for e in range(E)])
        # wd rows: expert-major local H rows -> [E*KL*128, D]
        wd_c = np.concatenate([td[e][hsl] for e in range(E)])
        wd_c = wd_c.reshape(E * KL, 128, DB, WSLAB).transpose(2, 1, 0, 3)
        in_maps.append({
            "wg": np.ascontiguousarray(
                wg_c.transpose(0, 2, 1, 3)).astype(NP_BF16),
            "wu": np.ascontiguousarray(
                wu_c.transpose(0, 2, 1, 3)).astype(NP_BF16),
            "wd": np.ascontiguousarray(wd_c).astype(NP_BF16),
            **xg_arrs,
        })
    fres = _run(fnc, in_maps, "ffn")
    if LAST_PHASE_NS:
        LAST_HW_NS = sum(LAST_PHASE_NS.values())

    # ---- unshard: sum partials over cores, then combine-weighted
    # scatter-add of the <=2 expert contributions per token ----
    out = np.zeros((N, D), dtype=np.float32)
    for e in range(E):
        nt, tsz = geoms[e]
        cap = nt * tsz
        acc = np.zeros((nt, 128, KO_D, tsz), dtype=np.float32)
        for c in range(NCORES):
            acc += fres.results[c][f"yt{e}"].astype(np.float32)
        yc = acc.transpose(0, 3, 2, 1).reshape(cap, D)
        out[toks[e]] += wts[e][:, None] * yc[:counts[e]]
    return out.reshape(B, T, D)


# revision 16
# speedup vs baseline: 1.0201x; 1.0201x over previous
"""Trainium2 Bass kernel for a top-2 ternary-weight MoE FFN — v3.

Sharding: tensor-parallel over the expert hidden dim H instead of
expert-parallel. Every core holds a 256-row H-slice of ALL 8 experts'
ternary weights and processes ALL ~16.4k token-expert pairs (grouped by
expert, each segment padded to a uniform per-expert tile size). This is
perfectly load-balanced for any routing skew: per-core PE work is
pairs x 48 cycles ~= 328 us, vs padding every core to the hottest
expert (2176 x 384 cycles = 348 us) under expert-parallel.

Host: fp64 routing (decision-exact vs the fp32 reference ordering),
ternarization (exact median threshold, {-1,0,+1} exact in bf16), pair
gather, DMA-friendly layouts. Unshard: sum the 8 partial y (each core's
down matmul contracts only its 256 H rows), then combine-weighted
scatter-add per token.
"""

import os

import numpy as np
import ml_dtypes

import concourse.bacc as bacc
import concourse.mybir as mybir
from concourse.tile import TileContext
from concourse.bass_utils import run_bass_kernel_spmd

FP32 = mybir.dt.float32
BF16 = mybir.dt.bfloat16
NP_BF16 = ml_dtypes.bfloat16

NCORES = 8
B, T, D, H, E = 4, 2048, 1024, 2048, 8
N = B * T                    # 8192 tokens
KO_D = 8                     # contraction chunks over D
HSH = H // NCORES            # 256 H rows per core per expert
KL = HSH // 128              # 2 local h chunks per expert
WSLAB = 256
DB = D // WSLAB              # 4 down-weight slabs

LAST_HW_NS = None
LAST_PHASE_NS = None

_program_cache = {}


def _ensure_ntff_hook():
    """Profiling-only: register the axon NTFF hook that the trimmed antenv
    package lacks, and stub out artifact upload (no bucket creds here)."""
    import sys
    import types

    import concourse.bass_utils as bu
    bu.upload_artifacts = lambda d: str(d)
    try:
        from antenv.axon_hooks import get_axon_ntff_profile_hook
        if get_axon_ntff_profile_hook() is not None:
            return
    except ImportError:
        mod = types.ModuleType("antenv.axon_hooks")
        box = {}
        mod.set_axon_ntff_profile_hook = lambda h: box.__setitem__("h", h)
        mod.get_axon_ntff_profile_hook = lambda: box.get("h")
        sys.modules["antenv.axon_hooks"] = mod
        import antenv
        antenv.axon_hooks = mod
    from antenv.axon_hooks import set_axon_ntff_profile_hook
    from trn_agent_boot.trn_boot import _ntff_profile_via_ctypes
    set_axon_ntff_profile_hook(
        _ntff_profile_via_ctypes("/opt/axon/libaxon_pjrt.so"))


def _run(nc, in_maps, label):
    trace = bool(int(os.environ.get("MOE_TRACE", "0")))
    kw = {}
    if trace:
        _ensure_ntff_hook()
        kw = dict(trace=True, trace_cores=list(range(NCORES)),
                  trace_kwargs={"title": label})
    res = run_bass_kernel_spmd(nc, in_maps, core_ids=list(range(NCORES)), **kw)
    if trace:
        global LAST_PHASE_NS
        print(f"[{label}] exec_time_ns={res.exec_time_ns} "
              f"mean={res.mean_exec_time_ns} "
              f"slowest_core={res.max_exec_time_core_id} "
              f"trace={res.instructions_and_trace[1] if res.instructions_and_trace else None}")
        if res.exec_time_ns:
            LAST_PHASE_NS[label] = res.exec_time_ns
    return res


def _tile_geom(mc):
    """Uniform tiles for one expert segment: nt tiles of tsz (<=512, x4)."""
    nt = max(1, -(-mc // 512))
    tsz = -(-mc // (nt * 4)) * 4
    return nt, tsz


def _build_ffn(geoms):
    """All-expert H-slice FFN. geoms[e] = (nt_e, tsz_e).

    DRAM layouts (per core, all bf16):
      wg/wu [E, 128, KO_D, 256]: [e, p, k, c] = tern(w_e).T[k*128+p, hsl+c]
        (hsl = this core's 256-column H-slice of expert e)
      wd    [E, 128, KL, D]: [e, p, l, dd] = tern(wd_e).T[hsl + l*128+p, dd]
      xg_e  [nt_e, 128, KO_D, tsz_e]: expert-e pair tokens, tiled
      yt_e  [nt_e, 128, KO_D, tsz_e]: partial outputs (sum over cores)
    """
    nc = bacc.Bacc("TRN2", target_bir_lowering=False, debug=False,
                   num_devices=NCORES)
    wg = nc.dram_tensor("wg", [E, 128, KO_D, WSLAB], BF16,
                        kind="ExternalInput")
    wu = nc.dram_tensor("wu", [E, 128, KO_D, WSLAB], BF16,
                        kind="ExternalInput")
    wd = nc.dram_tensor("wd", [E, 128, KL, D], BF16,
                        kind="ExternalInput")
    xg, yt = [], []
    for e, (nt, tsz) in enumerate(geoms):
        xg.append(nc.dram_tensor(f"xg{e}", [nt, 128, KO_D, tsz], BF16,
                                 kind="ExternalInput"))
        yt.append(nc.dram_tensor(f"yt{e}", [nt, 128, KO_D, tsz], BF16,
                                 kind="ExternalOutput"))

    with TileContext(nc) as tc:
        with (
            tc.tile_pool(name="wpool", bufs=1) as wpool,
            tc.tile_pool(name="xpool", bufs=2) as xpool,
            tc.tile_pool(name="mpool", bufs=2) as mpool,
            tc.tile_pool(name="spool", bufs=3) as spool,
            tc.tile_pool(name="ypool", bufs=4) as ypool,
            tc.tile_pool(name="ps_g", bufs=2, space="PSUM") as ps_g,
            tc.tile_pool(name="ps_u", bufs=2, space="PSUM") as ps_u,
            tc.tile_pool(name="ps_o", bufs=4, space="PSUM") as ps_o,
        ):
            # SBUF-resident H-slice weights: 96 KB/partition total.
            wg_sb = wpool.tile([128, E, KO_D, WSLAB], BF16)
            wu_sb = wpool.tile([128, E, KO_D, WSLAB], BF16)
            wd_sb = wpool.tile([128, E, KL, D], BF16)

            def load_xt(e, ti, tsz, chunked):
                xt_sb = xpool.tile([128, KO_D, 512], BF16, tag="xt")
                if chunked:  # very first tile: per-k DMAs to start sooner
                    for k in range(KO_D):
                        nc.sync.dma_start(xt_sb[:, k, :tsz],
                                          xg[e].ap()[ti, :, k, :])
                else:
                    nc.sync.dma_start(xt_sb[:, :, :tsz], xg[e].ap()[ti])
                return xt_sb

            # tokens on the sync queue; ALL weights on the SWDGE queue in
            # exact consumption order, fully expert-interleaved (wg_e, wu_e,
            # wd_e): expert e's 1.5 MiB arrives long before its ~40us
            # segment needs it, and wd_e lands before segment e's first
            # down matmul. First slabs halved so matmuls start sooner.
            xt_cur = load_xt(0, 0, geoms[0][1], True)
            nc.gpsimd.dma_start(wg_sb[:, 0, :, :128], wg.ap()[0, :, :, :128])
            nc.gpsimd.dma_start(wu_sb[:, 0, :, :128], wu.ap()[0, :, :, :128])
            nc.gpsimd.dma_start(wg_sb[:, 0, :, 128:], wg.ap()[0, :, :, 128:])
            nc.gpsimd.dma_start(wu_sb[:, 0, :, 128:], wu.ap()[0, :, :, 128:])
            nc.gpsimd.dma_start(wd_sb[:, 0], wd.ap()[0])
            for e in range(1, E):
                nc.gpsimd.dma_start(wg_sb[:, e], wg.ap()[e])
                nc.gpsimd.dma_start(wu_sb[:, e], wu.ap()[e])
                nc.gpsimd.dma_start(wd_sb[:, e], wd.ap()[e])
            drain_alt = 0
            for e in range(E):
                nt, tsz = geoms[e]
                for ti in range(nt):
                    m_sb = mpool.tile([128, KL, 512], BF16, tag="m")
                    for hl in range(KL):
                        pg = ps_g.tile([128, tsz], FP32, tag="pg")
                        pu = ps_u.tile([128, tsz], FP32, tag="pu")
                        csl = slice(hl * 128, (hl + 1) * 128)
                        for k in range(KO_D):
                            nc.tensor.matmul(pg[:], lhsT=wg_sb[:, e, k, csl],
                                             rhs=xt_cur[:, k, :tsz],
                                             start=(k == 0),
                                             stop=(k == KO_D - 1))
                        for k in range(KO_D):
                            nc.tensor.matmul(pu[:], lhsT=wu_sb[:, e, k, csl],
                                             rhs=xt_cur[:, k, :tsz],
                                             start=(k == 0),
                                             stop=(k == KO_D - 1))
                        sg = spool.tile([128, 512], BF16, tag="sg")
                        nc.scalar.activation(sg[:, :tsz], pg[:],
                                             mybir.ActivationFunctionType.Silu)
                        nc.vector.tensor_tensor(out=m_sb[:, hl, :tsz],
                                                in0=sg[:, :tsz], in1=pu[:],
                                                op=mybir.AluOpType.mult)
                    # prefetch next tile's tokens during the down matmuls
                    nxt = None
                    if ti + 1 < nt:
                        nxt = (e, ti + 1, tsz)
                    elif e + 1 < E:
                        nxt = (e + 1, 0, geoms[e + 1][1])
                    if nxt is not None:
                        xt_next = load_xt(nxt[0], nxt[1], nxt[2], False)
                    # whole-tile output staging: one store descriptor per
                    # tile keeps the SWDGE issue rate low (32 MiB total)
                    ysb = ypool.tile([128, KO_D, 512], BF16, tag="ysb")
                    for d in range(KO_D):
                        dsl = slice(d * 128, (d + 1) * 128)
                        po = ps_o.tile([128, tsz], FP32, tag="po")
                        for hl in range(KL):
                            nc.tensor.matmul(
                                po[:], lhsT=wd_sb[:, e, hl, dsl],
                                rhs=m_sb[:, hl, :tsz],
                                start=(hl == 0), stop=(hl == KL - 1))
                        # short accumulation chains drain faster than one
                        # engine can copy: alternate ACT / DVE
                        if drain_alt % 2 == 0:
                            nc.scalar.copy(ysb[:, d, :tsz], po[:])
                        else:
                            nc.vector.tensor_copy(ysb[:, d, :tsz], po[:])
                        drain_alt += 1
                    nc.gpsimd.dma_start(yt[e].ap()[ti], ysb[:, :, :tsz])
                    if nxt is not None:
                        xt_cur = xt_next
    nc.compile()
    return nc


def _get_program(key):
    if key not in _program_cache:
        _program_cache[key] = _build_ffn(key)
    return _program_cache[key]


def _ternary_t(w):
    """tern(w).T int8; exact median-of-|w| threshold."""
    w = np.ascontiguousarray(w, dtype=np.float32)
    med = np.median(np.abs(w))
    q = (w > med).astype(np.int8) - (w < -med).astype(np.int8)
    return np.ascontiguousarray(q.T)


def kernel(x, router_w, w_gate, w_up, w_down, top_k):
    assert int(top_k) == 2
    global LAST_HW_NS, LAST_PHASE_NS
    LAST_PHASE_NS = {}
    xf = np.ascontiguousarray(x.reshape(N, D).astype(np.float32))

    # ---- host routing (fp64 logits; top-2 ordering matches the fp32
    # reference, gaps are far above fp32 rounding noise) ----
    logits = xf.astype(np.float64) @ router_w.T.astype(np.float64)
    order = np.argsort(-logits, axis=1, kind="stable")
    e1 = order[:, 0]
    e2 = order[:, 1]
    ar = np.arange(N)
    w1 = 1.0 / (1.0 + np.exp(-(logits[ar, e1] - logits[ar, e2])))
    w2 = 1.0 - w1

    # ---- pair lists per expert ----
    toks, wts = [], []
    for e in range(E):
        sel = np.nonzero((e1 == e) | (e2 == e))[0]
        toks.append(sel)
        wts.append(np.where(e1[sel] == e, w1[sel], w2[sel]).astype(np.float32))
    counts = [len(s) for s in toks]
    geoms = tuple(_tile_geom(c) for c in counts)

    fnc = _get_program(geoms)
    xf_bf = xf.astype(NP_BF16)

    # per-expert token tiles (identical on every core)
    xg_arrs = {}
    for e in range(E):
        nt, tsz = geoms[e]
        cap = nt * tsz
        xgp = np.zeros((cap, D), dtype=NP_BF16)
        xgp[:counts[e]] = xf_bf[toks[e]]
        xg_arrs[f"xg{e}"] = np.ascontiguousarray(
            xgp.reshape(nt, tsz, KO_D, 128).transpose(0, 3, 2, 1))

    # per-core H-slice weight slabs
    tg = [_ternary_t(w_gate[e]) for e in range(E)]   # [D, H] int8
    tu = [_ternary_t(w_up[e]) for e in range(E)]
    td = [_ternary_t(w_down[e]) for e in range(E)]   # [H, D] int8
    in_maps = []
    for c in range(NCORES):
        hsl = slice(c * HSH, (c + 1) * HSH)
        wg_c = np.stack([tg[e][:, hsl].reshape(KO_D, 128, WSLAB)
                         for e in range(E)])          # [E, k, p, c]
        wu_c = np.stack([tu[e][:, hsl].reshape(KO_D, 128, WSLAB)
                         for e in range(E)])
        # wd: per expert [E, 128, KL, D]; [e, p, l, dd] = td[e][hsl][l*128+p, dd]
        wd_c = np.stack([td[e][hsl].reshape(KL, 128, D) for e in range(E)])
        wd_c = wd_c.transpose(0, 2, 1, 3)
        in_maps.append({
            "wg": np.ascontiguousarray(
                wg_c.transpose(0, 2, 1, 3)).astype(NP_BF16),
            "wu": np.ascontiguousarray(
                wu_c.transpose(0, 2, 1, 3)).astype(NP_BF16),
            "wd": np.ascontiguousarray(wd_c).astype(NP_BF16),
            **xg_arrs,
        })
    fres = _run(fnc, in_maps, "ffn")
    if LAST_PHASE_NS:
        LAST_HW_NS = sum(LAST_PHASE_NS.values())

    # ---- unshard: sum partials over cores, then combine-weighted
    # scatter-add of the <=2 expert contributions per token ----
    out = np.zeros((N, D), dtype=np.float32)
    for e in range(E):
        nt, tsz = geoms[e]
        cap = nt * tsz
        acc = np.zeros((nt, 128, KO_D, tsz), dtype=np.float32)
        for c in range(NCORES):
            acc += fres.results[c][f"yt{e}"].astype(np.float32)
        yc = acc.transpose(0, 3, 2, 1).reshape(cap, D)
        out[toks[e]] += wts[e][:, None] * yc[:counts[e]]
    return out.reshape(B, T, D)


# revision 17
# speedup vs baseline: 1.0233x; 1.0031x over previous
"""Trainium2 Bass kernel for a top-2 ternary-weight MoE FFN — v3.

Sharding: tensor-parallel over the expert hidden dim H instead of
expert-parallel. Every core holds a 256-row H-slice of ALL 8 experts'
ternary weights and processes ALL ~16.4k token-expert pairs (grouped by
expert, each segment padded to a uniform per-expert tile size). This is
perfectly load-balanced for any routing skew: per-core PE work is
pairs x 48 cycles ~= 328 us, vs padding every core to the hottest
expert (2176 x 384 cycles = 348 us) under expert-parallel.

Host: fp64 routing (decision-exact vs the fp32 reference ordering),
ternarization (exact median threshold, {-1,0,+1} exact in bf16), pair
gather, DMA-friendly layouts. Unshard: sum the 8 partial y (each core's
down matmul contracts only its 256 H rows), then combine-weighted
scatter-add per token.
"""

import os

import numpy as np
import ml_dtypes

import concourse.bacc as bacc
import concourse.mybir as mybir
from concourse.tile import TileContext
from concourse.bass_utils import run_bass_kernel_spmd

FP32 = mybir.dt.float32
BF16 = mybir.dt.bfloat16
NP_BF16 = ml_dtypes.bfloat16

NCORES = 8
B, T, D, H, E = 4, 2048, 1024, 2048, 8
N = B * T                    # 8192 tokens
KO_D = 8                     # contraction chunks over D
HSH = H // NCORES            # 256 H rows per core per expert
KL = HSH // 128              # 2 local h chunks per expert
WSLAB = 256
DB = D // WSLAB              # 4 down-weight slabs

LAST_HW_NS = None
LAST_PHASE_NS = None

_program_cache = {}


def _ensure_ntff_hook():
    """Profiling-only: register the axon NTFF hook that the trimmed antenv
    package lacks, and stub out artifact upload (no bucket creds here)."""
    import sys
    import types

    import concourse.bass_utils as bu
    bu.upload_artifacts = lambda d: str(d)
    try:
        from antenv.axon_hooks import get_axon_ntff_profile_hook
        if get_axon_ntff_profile_hook() is not None:
            return
    except ImportError:
        mod = types.ModuleType("antenv.axon_hooks")
        box = {}
        mod.set_axon_ntff_profile_hook = lambda h: box.__setitem__("h", h)
        mod.get_axon_ntff_profile_hook = lambda: box.get("h")
        sys.modules["antenv.axon_hooks"] = mod
        import antenv
        antenv.axon_hooks = mod
    from antenv.axon_hooks import set_axon_ntff_profile_hook
    from trn_agent_boot.trn_boot import _ntff_profile_via_ctypes
    set_axon_ntff_profile_hook(
        _ntff_profile_via_ctypes("/opt/axon/libaxon_pjrt.so"))


def _run(nc, in_maps, label):
    trace = bool(int(os.environ.get("MOE_TRACE", "0")))
    kw = {}
    if trace:
        _ensure_ntff_hook()
        kw = dict(trace=True, trace_cores=list(range(NCORES)),
                  trace_kwargs={"title": label})
    res = run_bass_kernel_spmd(nc, in_maps, core_ids=list(range(NCORES)), **kw)
    if trace:
        global LAST_PHASE_NS
        print(f"[{label}] exec_time_ns={res.exec_time_ns} "
              f"mean={res.mean_exec_time_ns} "
              f"slowest_core={res.max_exec_time_core_id} "
              f"trace={res.instructions_and_trace[1] if res.instructions_and_trace else None}")
        if res.exec_time_ns:
            LAST_PHASE_NS[label] = res.exec_time_ns
    return res


def _tile_geom(mc):
    """Uniform tiles for one expert segment: nt tiles of tsz (<=512, x4)."""
    nt = max(1, -(-mc // 512))
    tsz = -(-mc // (nt * 4)) * 4
    return nt, tsz


def _build_ffn(geoms):
    """All-expert H-slice FFN. geoms[e] = (nt_e, tsz_e).

    DRAM layouts (per core, all bf16):
      wg/wu [E, 128, KO_D, 256]: [e, p, k, c] = tern(w_e).T[k*128+p, hsl+c]
        (hsl = this core's 256-column H-slice of expert e)
      wd    [E, 128, KL, D]: [e, p, l, dd] = tern(wd_e).T[hsl + l*128+p, dd]
      xg_e  [nt_e, 128, KO_D, tsz_e]: expert-e pair tokens, tiled
      yt_e  [nt_e, 128, KO_D, tsz_e]: partial outputs (sum over cores)
    """
    nc = bacc.Bacc("TRN2", target_bir_lowering=False, debug=False,
                   num_devices=NCORES)
    wg = nc.dram_tensor("wg", [E, 128, KO_D, WSLAB], BF16,
                        kind="ExternalInput")
    wu = nc.dram_tensor("wu", [E, 128, KO_D, WSLAB], BF16,
                        kind="ExternalInput")
    wd = nc.dram_tensor("wd", [E, 128, KL, D], BF16,
                        kind="ExternalInput")
    xg, yt = [], []
    for e, (nt, tsz) in enumerate(geoms):
        xg.append(nc.dram_tensor(f"xg{e}", [nt, 128, KO_D, tsz], BF16,
                                 kind="ExternalInput"))
        yt.append(nc.dram_tensor(f"yt{e}", [nt, 128, KO_D, tsz], BF16,
                                 kind="ExternalOutput"))

    with TileContext(nc) as tc:
        with (
            tc.tile_pool(name="wpool", bufs=1) as wpool,
            tc.tile_pool(name="xpool", bufs=2) as xpool,
            tc.tile_pool(name="mpool", bufs=2) as mpool,
            tc.tile_pool(name="spool", bufs=3) as spool,
            tc.tile_pool(name="ypool", bufs=4) as ypool,
            tc.tile_pool(name="ps_g", bufs=2, space="PSUM") as ps_g,
            tc.tile_pool(name="ps_u", bufs=2, space="PSUM") as ps_u,
            tc.tile_pool(name="ps_o", bufs=4, space="PSUM") as ps_o,
        ):
            # SBUF-resident H-slice weights: 96 KB/partition total.
            wg_sb = wpool.tile([128, E, KO_D, WSLAB], BF16)
            wu_sb = wpool.tile([128, E, KO_D, WSLAB], BF16)
            wd_sb = wpool.tile([128, E, KL, D], BF16)

            def load_xt(e, ti, tsz, chunked):
                xt_sb = xpool.tile([128, KO_D, 512], BF16, tag="xt")
                if chunked:  # very first tile: per-k DMAs to start sooner
                    for k in range(KO_D):
                        nc.sync.dma_start(xt_sb[:, k, :tsz],
                                          xg[e].ap()[ti, :, k, :])
                else:
                    nc.sync.dma_start(xt_sb[:, :, :tsz], xg[e].ap()[ti])
                return xt_sb

            # tokens on the sync queue; ALL weights on the SWDGE queue in
            # exact consumption order, fully expert-interleaved (wg_e, wu_e,
            # wd_e): expert e's 1.5 MiB arrives long before its ~40us
            # segment needs it, and wd_e lands before segment e's first
            # down matmul. First slabs halved so matmuls start sooner.
            xt_cur = load_xt(0, 0, geoms[0][1], True)
            nc.gpsimd.dma_start(wg_sb[:, 0, :, :128], wg.ap()[0, :, :, :128])
            nc.gpsimd.dma_start(wu_sb[:, 0, :, :128], wu.ap()[0, :, :, :128])
            nc.gpsimd.dma_start(wg_sb[:, 0, :, 128:], wg.ap()[0, :, :, 128:])
            nc.gpsimd.dma_start(wu_sb[:, 0, :, 128:], wu.ap()[0, :, :, 128:])
            nc.gpsimd.dma_start(wd_sb[:, 0], wd.ap()[0])
            for e in range(1, E):
                nc.gpsimd.dma_start(wg_sb[:, e], wg.ap()[e])
                nc.gpsimd.dma_start(wu_sb[:, e], wu.ap()[e])
                nc.gpsimd.dma_start(wd_sb[:, e], wd.ap()[e])
            drain_alt = 0
            for e in range(E):
                nt, tsz = geoms[e]
                for ti in range(nt):
                    m_sb = mpool.tile([128, KL, 512], BF16, tag="m")
                    for hl in range(KL):
                        pg = ps_g.tile([128, tsz], FP32, tag="pg")
                        pu = ps_u.tile([128, tsz], FP32, tag="pu")
                        csl = slice(hl * 128, (hl + 1) * 128)
                        for k in range(KO_D):
                            nc.tensor.matmul(pg[:], lhsT=wg_sb[:, e, k, csl],
                                             rhs=xt_cur[:, k, :tsz],
                                             start=(k == 0),
                                             stop=(k == KO_D - 1))
                        for k in range(KO_D):
                            nc.tensor.matmul(pu[:], lhsT=wu_sb[:, e, k, csl],
                                             rhs=xt_cur[:, k, :tsz],
                                             start=(k == 0),
                                             stop=(k == KO_D - 1))
                        sg = spool.tile([128, 512], BF16, tag="sg")
                        nc.scalar.activation(sg[:, :tsz], pg[:],
                                             mybir.ActivationFunctionType.Silu)
                        nc.vector.tensor_tensor(out=m_sb[:, hl, :tsz],
                                                in0=sg[:, :tsz], in1=pu[:],
                                                op=mybir.AluOpType.mult)
                    # prefetch next tile's tokens during the down matmuls
                    nxt = None
                    if ti + 1 < nt:
                        nxt = (e, ti + 1, tsz)
                    elif e + 1 < E:
                        nxt = (e + 1, 0, geoms[e + 1][1])
                    if nxt is not None:
                        xt_next = load_xt(nxt[0], nxt[1], nxt[2], False)
                    # output stores ride the Scalar engine's own DMA queue:
                    # the SWDGE queue stays weights-only (so expert slabs
                    # land long before their segment) and the sync queue
                    # stays token-loads-only. Last tile stores per-d so the
                    # final flush after the last matmul is short.
                    last = (nxt is None)
                    ysb = ypool.tile([128, KO_D, 512], BF16, tag="ysb")
                    for d in range(KO_D):
                        dsl = slice(d * 128, (d + 1) * 128)
                        po = ps_o.tile([128, tsz], FP32, tag="po")
                        for hl in range(KL):
                            nc.tensor.matmul(
                                po[:], lhsT=wd_sb[:, e, hl, dsl],
                                rhs=m_sb[:, hl, :tsz],
                                start=(hl == 0), stop=(hl == KL - 1))
                        # short accumulation chains drain faster than one
                        # engine can copy: alternate ACT / DVE
                        if drain_alt % 2 == 0:
                            nc.scalar.copy(ysb[:, d, :tsz], po[:])
                        else:
                            nc.vector.tensor_copy(ysb[:, d, :tsz], po[:])
                        drain_alt += 1
                        if last:
                            nc.scalar.dma_start(yt[e].ap()[ti, :, d, :],
                                                ysb[:, d, :tsz])
                    if not last:
                        nc.scalar.dma_start(yt[e].ap()[ti], ysb[:, :, :tsz])
                        xt_cur = xt_next
    nc.compile()
    return nc


def _get_program(key):
    if key not in _program_cache:
        _program_cache[key] = _build_ffn(key)
    return _program_cache[key]


def _ternary_t(w):
    """tern(w).T int8; exact median-of-|w| threshold."""
    w = np.ascontiguousarray(w, dtype=np.float32)
    med = np.median(np.abs(w))
    q = (w > med).astype(np.int8) - (w < -med).astype(np.int8)
    return np.ascontiguousarray(q.T)


def kernel(x, router_w, w_gate, w_up, w_down, top_k):
    assert int(top_k) == 2
    global LAST_HW_NS, LAST_PHASE_NS
    LAST_PHASE_NS = {}
    xf = np.ascontiguousarray(x.reshape(N, D).astype(np.float32))

    # ---- host routing (fp64 logits; top-2 ordering matches the fp32
    # reference, gaps are far above fp32 rounding noise) ----
    logits = xf.astype(np.float64) @ router_w.T.astype(np.float64)
    order = np.argsort(-logits, axis=1, kind="stable")
    e1 = order[:, 0]
    e2 = order[:, 1]
    ar = np.arange(N)
    w1 = 1.0 / (1.0 + np.exp(-(logits[ar, e1] - logits[ar, e2])))
    w2 = 1.0 - w1

    # ---- pair lists per expert ----
    toks, wts = [], []
    for e in range(E):
        sel = np.nonzero((e1 == e) | (e2 == e))[0]
        toks.append(sel)
        wts.append(np.where(e1[sel] == e, w1[sel], w2[sel]).astype(np.float32))
    counts = [len(s) for s in toks]
    geoms = tuple(_tile_geom(c) for c in counts)

    fnc = _get_program(geoms)
    xf_bf = xf.astype(NP_BF16)

    # per-expert token tiles (identical on every core)
    xg_arrs = {}
    for e in range(E):
        nt, tsz = geoms[e]
        cap = nt * tsz
        xgp = np.zeros((cap, D), dtype=NP_BF16)
        xgp[:counts[e]] = xf_bf[toks[e]]
        xg_arrs[f"xg{e}"] = np.ascontiguousarray(
            xgp.reshape(nt, tsz, KO_D, 128).transpose(0, 3, 2, 1))

    # per-core H-slice weight slabs
    tg = [_ternary_t(w_gate[e]) for e in range(E)]   # [D, H] int8
    tu = [_ternary_t(w_up[e]) for e in range(E)]
    td = [_ternary_t(w_down[e]) for e in range(E)]   # [H, D] int8
    in_maps = []
    for c in range(NCORES):
        hsl = slice(c * HSH, (c + 1) * HSH)
        wg_c = np.stack([tg[e][:, hsl].reshape(KO_D, 128, WSLAB)
                         for e in range(E)])          # [E, k, p, c]
        wu_c = np.stack([tu[e][:, hsl].reshape(KO_D, 128, WSLAB)
                         for e in range(E)])
        # wd: per expert [E, 128, KL, D]; [e, p, l, dd] = td[e][hsl][l*128+p, dd]
        wd_c = np.stack([td[e][hsl].reshape(KL, 128, D) for e in range(E)])
        wd_c = wd_c.transpose(0, 2, 1, 3)
        in_maps.append({
            "wg": np.ascontiguousarray(
                wg_c.transpose(0, 2, 1, 3)).astype(NP_BF16),
            "wu": np.ascontiguousarray(
                wu_c.transpose(0, 2, 1, 3)).astype(NP_BF16),
            "wd": np.ascontiguousarray(wd_c).astype(NP_BF16),
            **xg_arrs,
        })
    fres = _run(fnc, in_maps, "ffn")
    if LAST_PHASE_NS:
        LAST_HW_NS = sum(LAST_PHASE_NS.values())

    # ---- unshard: sum partials over cores, then combine-weighted
    # scatter-add of the <=2 expert contributions per token ----
    out = np.zeros((N, D), dtype=np.float32)
    for e in range(E):
        nt, tsz = geoms[e]
        cap = nt * tsz
        acc = np.zeros((nt, 128, KO_D, tsz), dtype=np.float32)
        for c in range(NCORES):
            acc += fres.results[c][f"yt{e}"].astype(np.float32)
        yc = acc.transpose(0, 3, 2, 1).reshape(cap, D)
        out[toks[e]] += wts[e][:, None] * yc[:counts[e]]
    return out.reshape(B, T, D)


# revision 18
# speedup vs baseline: 1.0693x; 1.0450x over previous
"""Trainium2 Bass kernel for a top-2 ternary-weight MoE FFN.

Sharding: expert-parallel over 8 NeuronCores (1 expert/core), per the
expert-parallel hint. The router is a trivial 0.07%-of-FLOPs matmul, so
it is evaluated host-side in fp64 (decision-exact vs the fp32 reference
ordering) and the all-to-all is a host gather: each expert core receives
its routed token rows pre-transposed and pre-cast to bf16. Expert
weights are ternarized host-side (threshold = per-matrix median of |w|,
values {-1,0,+1} are exact in bf16) so the device program is a pure
bf16 3-matmul FFN stream: gate/up over D, silu*up, down over H. The
combine weights and the 2-way expert sum per token are applied during
the host unshard (a scaled scatter-add).

The device phase is PE-bound: 384 cycles/token at 2.4 GHz. Everything
else (weight/token DMA, silu on ACT, gate*up on DVE, PSUM drains) is
sized and queued to hide under the matmul stream.
"""

import math
import os

import numpy as np
import ml_dtypes

import concourse.bacc as bacc
import concourse.mybir as mybir
from concourse.tile import TileContext
from concourse.bass_utils import run_bass_kernel_spmd

FP32 = mybir.dt.float32
BF16 = mybir.dt.bfloat16
NP_BF16 = ml_dtypes.bfloat16

NCORES = 8
B, T, D, H, E = 4, 2048, 1024, 2048, 8
N = B * T                    # 8192 tokens
KO_D = D // 128              # 8 contraction chunks over D
KO_H = H // 128              # 16 contraction chunks over H

LAST_HW_NS = None
LAST_PHASE_NS = None

_program_cache = {}


def _ensure_ntff_hook():
    """Profiling-only: register the axon NTFF hook that the trimmed antenv
    package lacks, and stub out artifact upload (no bucket creds here)."""
    import sys
    import types

    import concourse.bass_utils as bu
    bu.upload_artifacts = lambda d: str(d)
    try:
        from antenv.axon_hooks import get_axon_ntff_profile_hook
        if get_axon_ntff_profile_hook() is not None:
            return
    except ImportError:
        mod = types.ModuleType("antenv.axon_hooks")
        box = {}
        mod.set_axon_ntff_profile_hook = lambda h: box.__setitem__("h", h)
        mod.get_axon_ntff_profile_hook = lambda: box.get("h")
        sys.modules["antenv.axon_hooks"] = mod
        import antenv
        antenv.axon_hooks = mod
    from antenv.axon_hooks import set_axon_ntff_profile_hook
    from trn_agent_boot.trn_boot import _ntff_profile_via_ctypes
    set_axon_ntff_profile_hook(
        _ntff_profile_via_ctypes("/opt/axon/libaxon_pjrt.so"))


def _run(nc, in_maps, label):
    trace = bool(int(os.environ.get("MOE_TRACE", "0")))
    kw = {}
    if trace:
        _ensure_ntff_hook()
        kw = dict(trace=True, trace_cores=list(range(NCORES)),
                  trace_kwargs={"title": label})
    res = run_bass_kernel_spmd(nc, in_maps, core_ids=list(range(NCORES)), **kw)
    if trace:
        global LAST_PHASE_NS
        print(f"[{label}] exec_time_ns={res.exec_time_ns} "
              f"mean={res.mean_exec_time_ns} "
              f"slowest_core={res.max_exec_time_core_id} "
              f"trace={res.instructions_and_trace[1] if res.instructions_and_trace else None}")
        if res.exec_time_ns:
            LAST_PHASE_NS[label] = res.exec_time_ns
    return res


WSLAB = 256                 # weight slab width (h / d columns per DMA)
HB = H // WSLAB             # 8 gate/up slabs
DB = D // WSLAB             # 4 down slabs


def _tile_geom(mc):
    """Uniform token tiles: nt tiles of tsz (<=512, multiple of 4)."""
    nt = max(1, -(-mc // 512))
    tsz = -(-mc // (nt * 4)) * 4
    return nt, tsz


def _build_ffn(nt, tsz):
    """Per-core expert FFN over nt*tsz gathered token rows, all bf16.

    Host pre-arranges every DRAM operand so each DMA moves >=4KB
    contiguous per partition:
      wg/wu [HB, 128, KO_D, WSLAB]: [j, p, k, c] = tern(w).T[k*128+p, j*256+c]
      wd    [DB, 128, KO_H, WSLAB]
      xg    [nt, 128, KO_D, tsz]:   [t, p, k, s] = x[tok t*tsz+s, k*128+p]
      yt    [nt, 128, KO_D, tsz]:   [t, p, d, s] = y[tok t*tsz+s, d*128+p]
    """
    nc = bacc.Bacc("TRN2", target_bir_lowering=False, debug=False,
                   num_devices=NCORES)
    wg = nc.dram_tensor("wg", [HB, 128, KO_D, WSLAB], BF16,
                        kind="ExternalInput")
    wu = nc.dram_tensor("wu", [HB, 128, KO_D, WSLAB], BF16,
                        kind="ExternalInput")
    wd = nc.dram_tensor("wd", [DB, 128, KO_H, WSLAB], BF16,
                        kind="ExternalInput")
    xg = nc.dram_tensor("xg", [nt, 128, KO_D, tsz], BF16,
                        kind="ExternalInput")
    yt = nc.dram_tensor("yt", [nt, 128, KO_D, tsz], BF16,
                        kind="ExternalOutput")

    with TileContext(nc) as tc:
        with (
            tc.tile_pool(name="wpool", bufs=1) as wpool,
            tc.tile_pool(name="xpool", bufs=2) as xpool,
            tc.tile_pool(name="mpool", bufs=2) as mpool,
            tc.tile_pool(name="spool", bufs=3) as spool,
            tc.tile_pool(name="ypool", bufs=3) as ypool,
            tc.tile_pool(name="ps_g", bufs=2, space="PSUM") as ps_g,
            tc.tile_pool(name="ps_u", bufs=2, space="PSUM") as ps_u,
            tc.tile_pool(name="ps_o", bufs=3, space="PSUM") as ps_o,
        ):
            # SBUF-resident ternary weights (bf16): 96 KB/partition total.
            wg_sb = wpool.tile([128, HB, KO_D, WSLAB], BF16)
            wu_sb = wpool.tile([128, HB, KO_D, WSLAB], BF16)
            wd_sb = wpool.tile([128, DB, KO_H, WSLAB], BF16)

            # All weight slabs on the SWDGE queue, in exact consumption
            # order for a gate-first tile 0 (wg0..wg7 then wu0..wu7): the
            # first tile computes ALL gate products first (needs only wg,
            # delivered 2x faster than consumed), silu-drains them to SBUF,
            # then the up pass runs against wu which has had a whole gate
            # pass worth of delivery time. Later tiles interleave normally.
            # First slab halved so the first matmul chain starts sooner.
            def load_xt(ti, chunked):
                xt_sb = xpool.tile([128, KO_D, tsz], BF16, tag="xt")
                if chunked:  # tile 0: per-k DMAs so matmuls start sooner
                    for k in range(KO_D):
                        nc.sync.dma_start(xt_sb[:, k, :], xg.ap()[ti, :, k, :])
                else:
                    nc.sync.dma_start(xt_sb[:], xg.ap()[ti])
                return xt_sb

            xt_first = load_xt(0, True)
            nc.gpsimd.dma_start(wg_sb[:, 0, :, :128], wg.ap()[0, :, :, :128])
            nc.gpsimd.dma_start(wg_sb[:, 0, :, 128:], wg.ap()[0, :, :, 128:])
            for j in range(1, HB):
                nc.gpsimd.dma_start(wg_sb[:, j], wg.ap()[j])
            for j in range(HB):
                nc.gpsimd.dma_start(wu_sb[:, j], wu.ap()[j])
            for j in range(DB):
                nc.gpsimd.dma_start(wd_sb[:, j], wd.ap()[j])

            def gu_lhsT(w_sb, hm):
                j, r = divmod(hm, 2)
                return w_sb[:, j, :, r * 128:(r + 1) * 128]

            xt_cur = xt_first
            for ti in range(nt):
                m_sb = mpool.tile([128, KO_H, tsz], BF16, tag="m")
                if ti == 0:
                    # two-pass tile 0: all-gate (silu into m_sb), then
                    # all-up multiplying m_sb in place
                    for hm in range(KO_H):
                        pg = ps_g.tile([128, tsz], FP32, tag="pg")
                        wg_l = gu_lhsT(wg_sb, hm)
                        for k in range(KO_D):
                            nc.tensor.matmul(pg[:], lhsT=wg_l[:, k, :],
                                             rhs=xt_cur[:, k, :],
                                             start=(k == 0),
                                             stop=(k == KO_D - 1))
                        nc.scalar.activation(m_sb[:, hm, :], pg[:],
                                             mybir.ActivationFunctionType.Silu)
                    for hm in range(KO_H):
                        pu = ps_u.tile([128, tsz], FP32, tag="pu")
                        wu_l = gu_lhsT(wu_sb, hm)
                        for k in range(KO_D):
                            nc.tensor.matmul(pu[:], lhsT=wu_l[:, k, :],
                                             rhs=xt_cur[:, k, :],
                                             start=(k == 0),
                                             stop=(k == KO_D - 1))
                        nc.vector.tensor_tensor(out=m_sb[:, hm, :],
                                                in0=m_sb[:, hm, :], in1=pu[:],
                                                op=mybir.AluOpType.mult)
                else:
                    for hm in range(KO_H):
                        pg = ps_g.tile([128, tsz], FP32, tag="pg")
                        pu = ps_u.tile([128, tsz], FP32, tag="pu")
                        wg_l = gu_lhsT(wg_sb, hm)
                        wu_l = gu_lhsT(wu_sb, hm)
                        for k in range(KO_D):
                            nc.tensor.matmul(pg[:], lhsT=wg_l[:, k, :],
                                             rhs=xt_cur[:, k, :],
                                             start=(k == 0),
                                             stop=(k == KO_D - 1))
                        for k in range(KO_D):
                            nc.tensor.matmul(pu[:], lhsT=wu_l[:, k, :],
                                             rhs=xt_cur[:, k, :],
                                             start=(k == 0),
                                             stop=(k == KO_D - 1))
                        sg = spool.tile([128, tsz], BF16, tag="sg")
                        nc.scalar.activation(sg[:], pg[:],
                                             mybir.ActivationFunctionType.Silu)
                        nc.vector.tensor_tensor(out=m_sb[:, hm, :], in0=sg[:],
                                                in1=pu[:],
                                                op=mybir.AluOpType.mult)
                # prefetch next tile's tokens while the down matmuls run
                if ti + 1 < nt:
                    xt_next = load_xt(ti + 1, False)
                for d in range(KO_D):
                    j, r = divmod(d, 2)
                    wd_l = wd_sb[:, j, :, r * 128:(r + 1) * 128]
                    po = ps_o.tile([128, tsz], FP32, tag="po")
                    for hm in range(KO_H):
                        nc.tensor.matmul(po[:], lhsT=wd_l[:, hm, :],
                                         rhs=m_sb[:, hm, :],
                                         start=(hm == 0), stop=(hm == KO_H - 1))
                    ysb = ypool.tile([128, tsz], BF16, tag="ysb")
                    nc.scalar.copy(ysb[:], po[:])
                    # per-d stores on the Scalar engine's own DMA queue so
                    # the SWDGE queue stays weights-only and the sync queue
                    # stays token-loads-only
                    nc.scalar.dma_start(yt.ap()[ti, :, d, :], ysb[:])
                if ti + 1 < nt:
                    xt_cur = xt_next
    nc.compile()
    return nc


def _get_program(key):
    if key not in _program_cache:
        _program_cache[key] = _build_ffn(*key)
    return _program_cache[key]


def _ternary_slabs(w, ko):
    """tern(w).T rearranged to [cols/256, 128, ko, 256] DMA-slab layout;
    exact median-of-|w| threshold and exact {-1,0,+1} values in bf16."""
    w = np.ascontiguousarray(w, dtype=np.float32)
    med = np.median(np.abs(w))
    q = (w > med).astype(np.int8) - (w < -med).astype(np.int8)
    qt = np.ascontiguousarray(q.T)              # [ko*128, cols]
    cols = qt.shape[1]
    r = qt.reshape(ko, 128, cols // WSLAB, WSLAB).transpose(2, 1, 0, 3)
    return np.ascontiguousarray(r).astype(NP_BF16)


def kernel(x, router_w, w_gate, w_up, w_down, top_k):
    assert int(top_k) == 2
    global LAST_HW_NS, LAST_PHASE_NS
    LAST_PHASE_NS = {}
    xf = np.ascontiguousarray(x.reshape(N, D).astype(np.float32))

    # ---- host routing (fp64 logits; top-2 ordering matches the fp32
    # reference, gaps are far above fp32 rounding noise) ----
    logits = xf.astype(np.float64) @ router_w.T.astype(np.float64)
    order = np.argsort(-logits, axis=1, kind="stable")
    e1 = order[:, 0]
    e2 = order[:, 1]
    ar = np.arange(N)
    # normalized top-2 softmax weights: w1 = sigmoid(l1 - l2)
    w1 = 1.0 / (1.0 + np.exp(-(logits[ar, e1] - logits[ar, e2])))
    w2 = 1.0 - w1

    # ---- host all-to-all: token rows -> expert cores ----
    toks, wts = [], []
    for e in range(E):
        sel = np.nonzero((e1 == e) | (e2 == e))[0]
        toks.append(sel)
        wts.append(np.where(e1[sel] == e, w1[sel], w2[sel]).astype(np.float32))
    counts = [len(s) for s in toks]
    nt, tsz = _tile_geom(max(max(counts), 512))
    cap = nt * tsz

    fnc = _get_program((nt, tsz))
    xf_bf = xf.astype(NP_BF16)
    in_maps = []
    for e in range(E):
        xgp = np.zeros((cap, D), dtype=NP_BF16)
        xgp[:counts[e]] = xf_bf[toks[e]]
        # [cap, D] -> [nt, 128, KO_D, tsz] DMA-tile layout
        xg = xgp.reshape(nt, tsz, KO_D, 128).transpose(0, 3, 2, 1)
        in_maps.append({
            "wg": _ternary_slabs(w_gate[e], KO_D),
            "wu": _ternary_slabs(w_up[e], KO_D),
            "wd": _ternary_slabs(w_down[e], KO_H),
            "xg": np.ascontiguousarray(xg),
        })
    fres = _run(fnc, in_maps, "ffn")
    if LAST_PHASE_NS:
        LAST_HW_NS = sum(LAST_PHASE_NS.values())

    # ---- unshard: combine-weighted sum of the <=2 expert outputs/token ----
    out = np.zeros((N, D), dtype=np.float32)
    for e in range(E):
        # yt [nt, 128, KO_D, tsz] -> [cap, D]
        yc = fres.results[e]["yt"].transpose(0, 3, 2, 1).reshape(cap, D)
        out[toks[e]] += wts[e][:, None] * yc[:counts[e]].astype(np.float32)
    return out.reshape(B, T, D)
